# revision 5
# baseline (speedup 1.0000x reference)
"""MAGAT GNN message-passing kernel for 8 Trainium2 NeuronCores.

Math: the reference applies Sinkhorn-Knopp to adj0 but only ever uses the
result via `adj > 0` — and Sinkhorn preserves the zero/positive pattern
exactly in fp32 (0/s == 0, pos/pos can't underflow at these magnitudes).
So the device kernel skips Sinkhorn and uses (adj0 > 0) as the softmax
mask (adj0 is shipped to the device as bf16, which also preserves the
zero/positive pattern exactly and halves the DMA traffic).

exp(leaky_relu(e)) with e = e_src[i] + e_dst[j] factors into rank-1
products: exp(e) = exp(e_src)*exp(e_dst) and exp(.2e) likewise, and
exp(leaky(e)) = max(exp(e), exp(.2e)) since exp is monotone. So no
per-element transcendental is needed — the steady state is two bf16 DVE
ops (running in 2x perf mode) plus one ACT broadcast-multiply per chunk.
Softmax runs without max-subtraction (e bounded by ~±4) and the row-sum
is fused into the attention matmul as a ones-column. The matmul runs in
bf16: the residual x0 (O(1)) dominates h_prime (O(0.01)), so bf16
rounding perturbs the final output by only ~1e-4 relative.

Sharding: 8 cores = 4 heads x 2 row-halves. Each core gets its head's
adjacency slice pre-transposed on host to [j=4096, i=2048] so the softmax
reduction over j lands on the PE contraction (partition) axis. x0 is
rolled per-core so "own rows" are always rows 0..2048 — keeps the SPMD
program identical across cores.
"""

import numpy as np
import ml_dtypes
from contextlib import ExitStack

import concourse.bacc as bacc
import concourse.mybir as mybir
import concourse.tile as tile
import concourse.masks as masks
from concourse.bass_utils import run_bass_kernel_spmd

F32 = mybir.dt.float32
BF16 = mybir.dt.bfloat16
N, F, H, D = 4096, 128, 4, 128
NH = N // 2          # own rows per core
NC = N // 128        # 32 j-chunks
IPASS = 2            # i splits (PSUM capacity: 8 banks of [128,129])
IW = NH // IPASS     # 1024 i per pass
ALPHA = 0.2

_cache = {}


def _build():
    nc = bacc.Bacc("TRN2", target_bir_lowering=False, debug=False)
    adjT = nc.dram_tensor("adjT", [N, NH], BF16, kind="ExternalInput").ap()
    x0r = nc.dram_tensor("x0r", [N, F], F32, kind="ExternalInput").ap()
    w = nc.dram_tensor("w", [F, D], F32, kind="ExternalInput").ap()
    asrc = nc.dram_tensor("asrc", [D, 1], F32, kind="ExternalInput").ap()
    adst = nc.dram_tensor("adst", [D, 1], F32, kind="ExternalInput").ap()
    out = nc.dram_tensor("out", [NH, D], F32, kind="ExternalOutput").ap()

    with tile.TileContext(nc) as tc, ExitStack() as ctx:
        const = ctx.enter_context(tc.tile_pool(name="const", bufs=1))

        # persistent tiles
        x0_sb = const.tile([128, NC * F], F32)        # x0 rows chunked [p, c, f]
        x03 = x0_sb[:].rearrange("p (c f) -> p c f", c=NC)
        whp = const.tile([128, NC * (D + 1)], BF16)   # [Wh | 1] per j-chunk, bf16
        whp3 = whp[:].rearrange("p (c q) -> p c q", c=NC)
        eA = const.tile([128, NH], BF16)              # exp(e_src) bcast
        ea = const.tile([128, NH], BF16)              # exp(0.2*e_src) bcast
        eB = const.tile([128, NC], F32)               # exp(e_dst)
        eb = const.tile([128, NC], F32)               # exp(0.2*e_dst)

        with ExitStack() as sctx:
            setup = sctx.enter_context(tc.tile_pool(name="setup", bufs=2))
            spsum = sctx.enter_context(tc.tile_pool(name="spsum", bufs=2, space="PSUM"))

            ident = setup.tile([128, 128], F32)
            masks.make_identity(nc, ident[:])
            w_sb = setup.tile([F, D], F32)
            nc.sync.dma_start(w_sb[:], w)
            asrc_sb = setup.tile([D, 1], F32)
            nc.sync.dma_start(asrc_sb[:], asrc)
            adst_sb = setup.tile([D, 1], F32)
            nc.sync.dma_start(adst_sb[:], adst)

            nc.sync.dma_start(
                x03[:, :, :], x0r.rearrange("(c p) f -> p c f", p=128))

            # x0T[f, n] via PE transpose per 128-chunk
            x0T = setup.tile([128, N], F32)
            for c in range(NC):
                pst = spsum.tile([128, 128], F32, tag="sps", name="pst")
                nc.tensor.transpose(pst[:], x03[:, c, :], ident[:])
                nc.scalar.copy(x0T[:, c * 128:(c + 1) * 128], pst[:])

            # Wh chunks -> whp cols 0..128 (cast to bf16); ones col at 128
            for c in range(NC):
                psw = spsum.tile([128, D], F32, tag="sps", name="psw")
                nc.tensor.matmul(psw[:], lhsT=x0T[:, c * 128:(c + 1) * 128],
                                 rhs=w_sb[:], start=True, stop=True)
                nc.vector.tensor_copy(whp3[:, c, 0:D], psw[:])
            nc.vector.memset(whp3[:, :, D], 1.0)

            # WhT[d, n]
            whT = setup.tile([128, N], F32)
            for g in range(N // 512):
                psq = spsum.tile([128, 512], F32, tag="sps", name="psq")
                nc.tensor.matmul(psq[:], lhsT=w_sb[:],
                                 rhs=x0T[:, g * 512:(g + 1) * 512],
                                 start=True, stop=True)
                nc.scalar.copy(whT[:, g * 512:(g + 1) * 512], psq[:])

            # e_src (own rows only) as a [1, NH] row
            es_row = setup.tile([1, NH], F32)
            for g in range(NH // 512):
                pse = spsum.tile([1, 512], F32, tag="sps", name="pse")
                nc.tensor.matmul(pse[:], lhsT=asrc_sb[:],
                                 rhs=whT[:, g * 512:(g + 1) * 512],
                                 start=True, stop=True)
                nc.vector.tensor_copy(es_row[:, g * 512:(g + 1) * 512], pse[:])

            # e_dst per j-chunk -> ed_sb[:, c]
            ed_sb = setup.tile([128, NC], F32)
            for c in range(NC):
                psd = spsum.tile([128, 1], F32, tag="sps", name="psd")
                nc.tensor.matmul(psd[:], lhsT=whT[:, c * 128:(c + 1) * 128],
                                 rhs=adst_sb[:], start=True, stop=True)
                nc.vector.tensor_copy(ed_sb[:, c:c + 1], psd[:])

            # esb = broadcast es_row across 128 partitions (ones ⊗ es_row)
            esb = setup.tile([128, NH], F32)
            ones_row = setup.tile([1, 128], F32)
            nc.vector.memset(ones_row[:], 1.0)
            for g in range(NH // 512):
                psb = spsum.tile([128, 512], F32, tag="sps", name="psb")
                nc.tensor.matmul(psb[:], lhsT=ones_row[:],
                                 rhs=es_row[:, g * 512:(g + 1) * 512],
                                 start=True, stop=True)
                nc.scalar.copy(esb[:, g * 512:(g + 1) * 512], psb[:])

            # rank-1 exp factors
            nc.scalar.activation(eA[:], esb[:], mybir.ActivationFunctionType.Exp)
            nc.scalar.activation(ea[:], esb[:], mybir.ActivationFunctionType.Exp,
                                 scale=0.2)
            nc.scalar.activation(eB[:], ed_sb[:], mybir.ActivationFunctionType.Exp)
            nc.scalar.activation(eb[:], ed_sb[:], mybir.ActivationFunctionType.Exp,
                                 scale=0.2)

        # steady state
        work = ctx.enter_context(tc.tile_pool(name="work", bufs=3))
        atp = ctx.enter_context(tc.tile_pool(name="atp", bufs=6))
        epil = ctx.enter_context(tc.tile_pool(name="epil", bufs=2))
        mpsum = ctx.enter_context(tc.tile_pool(name="mpsum", bufs=1, space="PSUM"))

        for ip in range(IPASS):
            iw = slice(ip * IW, (ip + 1) * IW)
            pss = [mpsum.tile([128, D + 1], F32, tag=f"acc{m}", name=f"acc_{ip}_{m}")
                   for m in range(8)]
            for jc in range(NC):
                at = atp.tile([128, IW], BF16, tag="at")
                nc.sync.dma_start(at[:], adjT[jc * 128:(jc + 1) * 128, iw])
                # t = exp(e_src)*exp(e_dst[jc])   (the e>0 branch of leaky)
                t = work.tile([128, IW], BF16, tag="t")
                nc.scalar.mul(t[:], eA[:, iw], eB[:, jc:jc + 1])
                # p1 = max(t, exp(.2 e_src)*exp(.2 e_dst[jc]))
                p1 = work.tile([128, IW], BF16, tag="p1")
                nc.vector.scalar_tensor_tensor(
                    out=p1[:], in0=ea[:, iw], scalar=eb[:, jc:jc + 1], in1=t[:],
                    op0=mybir.AluOpType.mult, op1=mybir.AluOpType.max)
                # pm = (adjT > 0) * p1
                pm = work.tile([128, IW], BF16, tag="pm")
                nc.vector.scalar_tensor_tensor(
                    out=pm[:], in0=at[:], scalar=0.0, in1=p1[:],
                    op0=mybir.AluOpType.is_gt, op1=mybir.AluOpType.mult)
                for m in range(8):
                    nc.tensor.matmul(pss[m][:], lhsT=pm[:, m * 128:(m + 1) * 128],
                                     rhs=whp3[:, jc, :],
                                     start=(jc == 0), stop=(jc == NC - 1))

            for m in range(8):
                ps = pss[m]
                rec = epil.tile([128, 1], F32, tag="rec")
                nc.vector.reciprocal(rec[:], ps[:, D:D + 1])
                hp = epil.tile([128, D], F32, tag="hp")
                nc.scalar.mul(hp[:], ps[:, 0:D], rec[:])
                # elu(x) = max(x, exp(min(x,0)) - 1)
                t1 = epil.tile([128, D], F32, tag="t1")
                nc.vector.tensor_scalar_min(t1[:], hp[:], 0.0)
                ex1 = epil.tile([128, D], F32, tag="ex1")
                nc.scalar.activation(ex1[:], t1[:],
                                     mybir.ActivationFunctionType.Exp)
                el1 = epil.tile([128, D], F32, tag="el1")
                nc.vector.scalar_tensor_tensor(
                    out=el1[:], in0=ex1[:], scalar=-1.0, in1=hp[:],
                    op0=mybir.AluOpType.add, op1=mybir.AluOpType.max)
                # residual + second elu
                r = epil.tile([128, D], F32, tag="r")
                nc.vector.tensor_add(r[:], el1[:], x03[:, ip * 8 + m, :])
                t2 = epil.tile([128, D], F32, tag="t2")
                nc.vector.tensor_scalar_min(t2[:], r[:], 0.0)
                ex2 = epil.tile([128, D], F32, tag="ex2")
                nc.scalar.activation(ex2[:], t2[:],
                                     mybir.ActivationFunctionType.Exp)
                y = epil.tile([128, D], F32, tag="y")
                nc.vector.scalar_tensor_tensor(
                    out=y[:], in0=ex2[:], scalar=-1.0, in1=r[:],
                    op0=mybir.AluOpType.add, op1=mybir.AluOpType.max)
                row0 = (ip * 8 + m) * 128
                nc.sync.dma_start(out[row0:row0 + 128, :], y[:])

    nc.compile()
    return nc


def _get_nc():
    if "nc" not in _cache:
        _cache["nc"] = _build()
    return _cache["nc"]


def kernel(x0, adj0, W, a_src, a_dst):
    nc = _get_nc()
    in_maps = []
    for c in range(8):
        h, half = c // 2, c % 2
        i0 = half * NH
        a = adj0[h, i0:i0 + NH, :]
        if i0:
            a = np.concatenate([a[:, i0:], a[:, :i0]], axis=1)
            xr = np.concatenate([x0[i0:], x0[:i0]], axis=0)
        else:
            xr = x0
        in_maps.append(dict(
            adjT=np.ascontiguousarray(a.T).astype(ml_dtypes.bfloat16),
            x0r=np.ascontiguousarray(xr),
            w=np.ascontiguousarray(W[h]),
            asrc=np.ascontiguousarray(a_src[h][:, None]),
            adst=np.ascontiguousarray(a_dst[h][:, None]),
        ))
    res = run_bass_kernel_spmd(nc, in_maps, core_ids=list(range(8))).results
    x1 = np.empty((N, H * D), np.float32)
    for c in range(8):
        h, half = c // 2, c % 2
        i0 = half * NH
        x1[i0:i0 + NH, h * D:(h + 1) * D] = res[c]["out"]
    return x1


# revision 6
# speedup vs baseline: 1.1407x; 1.1407x over previous
"""MAGAT GNN message-passing kernel for 8 Trainium2 NeuronCores.

Math: the reference applies Sinkhorn-Knopp to adj0 but only ever uses the
result via `adj > 0` — and Sinkhorn preserves the zero/positive pattern
exactly in fp32 (0/s == 0, pos/pos can't underflow at these magnitudes).
So the device kernel skips Sinkhorn and uses (adj0 > 0) as the softmax
mask (adj0 is shipped to the device as bf16, which also preserves the
zero/positive pattern exactly and halves the DMA traffic).

exp(leaky_relu(e)) with e = e_src[i] + e_dst[j] factors into rank-1
products: exp(e) = exp(e_src)*exp(e_dst) and exp(.2e) likewise, and
exp(leaky(e)) = max(exp(e), exp(.2e)) since exp is monotone. So no
per-element transcendental is needed — the steady state is two bf16 DVE
ops (running in 2x perf mode) plus one ACT broadcast-multiply per chunk.
Softmax runs without max-subtraction (e bounded by ~±4) and the row-sum
is fused into the attention matmul as a ones-column. The matmul runs in
bf16: the residual x0 (O(1)) dominates h_prime (O(0.01)), so bf16
rounding perturbs the final output by only ~1e-4 relative.

Sharding: 8 cores = 4 heads x 2 row-halves. Each core gets its head's
adjacency slice pre-transposed on host to [j=4096, i=2048] so the softmax
reduction over j lands on the PE contraction (partition) axis. x0 is
rolled per-core so "own rows" are always rows 0..2048 — keeps the SPMD
program identical across cores.
"""

import numpy as np
import ml_dtypes
from contextlib import ExitStack

import concourse.bacc as bacc
import concourse.mybir as mybir
import concourse.tile as tile
import concourse.masks as masks
from concourse.bass_utils import run_bass_kernel_spmd

F32 = mybir.dt.float32
BF16 = mybir.dt.bfloat16
N, F, H, D = 4096, 128, 4, 128
NH = N // 2          # own rows per core
NC = N // 128        # 32 j-chunks
IPASS = 2            # i splits (PSUM capacity: 8 banks of [128,129])
IW = NH // IPASS     # 1024 i per pass
ALPHA = 0.2

_cache = {}


def _build():
    nc = bacc.Bacc("TRN2", target_bir_lowering=False, debug=False)
    adjT = nc.dram_tensor("adjT", [N, NH], BF16, kind="ExternalInput").ap()
    x0r = nc.dram_tensor("x0r", [N, F], F32, kind="ExternalInput").ap()
    w = nc.dram_tensor("w", [F, D], F32, kind="ExternalInput").ap()
    asrc = nc.dram_tensor("asrc", [D, 1], F32, kind="ExternalInput").ap()
    adst = nc.dram_tensor("adst", [D, 1], F32, kind="ExternalInput").ap()
    out = nc.dram_tensor("out", [NH, D], F32, kind="ExternalOutput").ap()

    with tile.TileContext(nc) as tc, ExitStack() as ctx:
        const = ctx.enter_context(tc.tile_pool(name="const", bufs=1))

        # persistent tiles
        x0_sb = const.tile([128, NC * F], F32)        # x0 rows chunked [p, c, f]
        x03 = x0_sb[:].rearrange("p (c f) -> p c f", c=NC)
        whp = const.tile([128, NC * (D + 1)], BF16)   # [Wh | 1] per j-chunk, bf16
        whp3 = whp[:].rearrange("p (c q) -> p c q", c=NC)
        eA = const.tile([128, NH], BF16)              # exp(e_src) bcast
        ea = const.tile([128, NH], BF16)              # exp(0.2*e_src) bcast
        eB = const.tile([128, NC], F32)               # exp(e_dst)
        eb = const.tile([128, NC], F32)               # exp(0.2*e_dst)
        esb = const.tile([128, NH], F32)              # e_src bcast (f32)
        ed_sb = const.tile([128, NC], F32)            # e_dst per chunk

        with ExitStack() as sctx:
            setup = sctx.enter_context(tc.tile_pool(name="setup", bufs=2))
            spsum = sctx.enter_context(tc.tile_pool(name="spsum", bufs=2, space="PSUM"))

            ident = setup.tile([128, 128], F32)
            masks.make_identity(nc, ident[:])
            w_sb = setup.tile([F, D], F32)
            nc.sync.dma_start(w_sb[:], w)
            asrc_sb = setup.tile([D, 1], F32)
            nc.sync.dma_start(asrc_sb[:], asrc)
            adst_sb = setup.tile([D, 1], F32)
            nc.sync.dma_start(adst_sb[:], adst)

            nc.sync.dma_start(
                x03[:, :, :], x0r.rearrange("(c p) f -> p c f", p=128))

            # x0T[f, n] via PE transpose per 128-chunk
            x0T = setup.tile([128, N], F32)
            for c in range(NC):
                pst = spsum.tile([128, 128], F32, tag="sps", name="pst")
                nc.tensor.transpose(pst[:], x03[:, c, :], ident[:])
                nc.scalar.copy(x0T[:, c * 128:(c + 1) * 128], pst[:])

            # Wh chunks -> whp cols 0..128 (cast to bf16); ones col at 128
            for c in range(NC):
                psw = spsum.tile([128, D], F32, tag="sps", name="psw")
                nc.tensor.matmul(psw[:], lhsT=x0T[:, c * 128:(c + 1) * 128],
                                 rhs=w_sb[:], start=True, stop=True)
                nc.vector.tensor_copy(whp3[:, c, 0:D], psw[:])
            nc.vector.memset(whp3[:, :, D], 1.0)

            # WhT[d, n]
            whT = setup.tile([128, N], F32)
            for g in range(N // 512):
                psq = spsum.tile([128, 512], F32, tag="sps", name="psq")
                nc.tensor.matmul(psq[:], lhsT=w_sb[:],
                                 rhs=x0T[:, g * 512:(g + 1) * 512],
                                 start=True, stop=True)
                nc.scalar.copy(whT[:, g * 512:(g + 1) * 512], psq[:])

            # e_src (own rows only) as a [1, NH] row
            es_row = setup.tile([1, NH], F32)
            for g in range(NH // 512):
                pse = spsum.tile([1, 512], F32, tag="sps", name="pse")
                nc.tensor.matmul(pse[:], lhsT=asrc_sb[:],
                                 rhs=whT[:, g * 512:(g + 1) * 512],
                                 start=True, stop=True)
                nc.vector.tensor_copy(es_row[:, g * 512:(g + 1) * 512], pse[:])

            # e_dst per j-chunk -> ed_sb[:, c]
            for c in range(NC):
                psd = spsum.tile([128, 1], F32, tag="sps", name="psd")
                nc.tensor.matmul(psd[:], lhsT=whT[:, c * 128:(c + 1) * 128],
                                 rhs=adst_sb[:], start=True, stop=True)
                nc.vector.tensor_copy(ed_sb[:, c:c + 1], psd[:])

            # esb = broadcast es_row across 128 partitions (ones ⊗ es_row)
            ones_row = setup.tile([1, 128], F32)
            nc.vector.memset(ones_row[:], 1.0)
            for g in range(NH // 512):
                psb = spsum.tile([128, 512], F32, tag="sps", name="psb")
                nc.tensor.matmul(psb[:], lhsT=ones_row[:],
                                 rhs=es_row[:, g * 512:(g + 1) * 512],
                                 start=True, stop=True)
                nc.scalar.copy(esb[:, g * 512:(g + 1) * 512], psb[:])

            # rank-1 exp factors
            nc.scalar.activation(eA[:], esb[:], mybir.ActivationFunctionType.Exp)
            nc.scalar.activation(ea[:], esb[:], mybir.ActivationFunctionType.Exp,
                                 scale=0.2)
            nc.scalar.activation(eB[:], ed_sb[:], mybir.ActivationFunctionType.Exp)
            nc.scalar.activation(eb[:], ed_sb[:], mybir.ActivationFunctionType.Exp,
                                 scale=0.2)

        # steady state
        work = ctx.enter_context(tc.tile_pool(name="work", bufs=3))
        atp = ctx.enter_context(tc.tile_pool(name="atp", bufs=6))
        epil = ctx.enter_context(tc.tile_pool(name="epil", bufs=2))
        mpsum = ctx.enter_context(tc.tile_pool(name="mpsum", bufs=1, space="PSUM"))

        for ip in range(IPASS):
            iw = slice(ip * IW, (ip + 1) * IW)
            pss = [mpsum.tile([128, D + 1], F32, tag=f"acc{m}", name=f"acc_{ip}_{m}")
                   for m in range(8)]
            for jc in range(NC):
                at = atp.tile([128, IW], BF16, tag="at")
                nc.sync.dma_start(at[:], adjT[jc * 128:(jc + 1) * 128, iw])
                if (jc % 5) in (1, 3):
                    # cfgB rank-1: t = exp(e_src)*exp(e_dst[jc]) on ACT,
                    # max with exp(.2e) branch on DVE
                    t = work.tile([128, IW], BF16, tag="t")
                    nc.scalar.mul(t[:], eA[:, iw], eB[:, jc:jc + 1])
                    p1 = work.tile([128, IW], BF16, tag="p1")
                    nc.vector.scalar_tensor_tensor(
                        out=p1[:], in0=ea[:, iw], scalar=eb[:, jc:jc + 1], in1=t[:],
                        op0=mybir.AluOpType.mult, op1=mybir.AluOpType.max)
                else:
                    # cfgA: leaky-relu then exp, both on ACT
                    el = work.tile([128, IW], F32, tag="el")
                    nc.scalar.activation(el[:], esb[:, iw],
                                         mybir.ActivationFunctionType.Prelu,
                                         bias=ed_sb[:, jc:jc + 1], scale=1.0,
                                         alpha=ALPHA)
                    p1 = work.tile([128, IW], BF16, tag="p1")
                    nc.scalar.activation(p1[:], el[:],
                                         mybir.ActivationFunctionType.Exp)
                # pm = (adjT > 0) * p1
                pm = work.tile([128, IW], BF16, tag="pm")
                nc.vector.scalar_tensor_tensor(
                    out=pm[:], in0=at[:], scalar=0.0, in1=p1[:],
                    op0=mybir.AluOpType.is_gt, op1=mybir.AluOpType.mult)
                for m in range(8):
                    nc.tensor.matmul(pss[m][:], lhsT=pm[:, m * 128:(m + 1) * 128],
                                     rhs=whp3[:, jc, :],
                                     start=(jc == 0), stop=(jc == NC - 1))

            for m in range(8):
                ps = pss[m]
                rec = epil.tile([128, 1], F32, tag="rec")
                nc.vector.reciprocal(rec[:], ps[:, D:D + 1])
                hp = epil.tile([128, D], F32, tag="hp")
                nc.scalar.mul(hp[:], ps[:, 0:D], rec[:])
                # elu(x) = max(x, exp(min(x,0)) - 1); min(x,0) = -relu(-x)
                t1 = epil.tile([128, D], F32, tag="t1")
                nc.scalar.activation(t1[:], hp[:],
                                     mybir.ActivationFunctionType.Relu,
                                     scale=-1.0)
                ex1 = epil.tile([128, D], F32, tag="ex1")
                nc.scalar.activation(ex1[:], t1[:],
                                     mybir.ActivationFunctionType.Exp,
                                     scale=-1.0)
                el1 = epil.tile([128, D], F32, tag="el1")
                nc.vector.scalar_tensor_tensor(
                    out=el1[:], in0=ex1[:], scalar=-1.0, in1=hp[:],
                    op0=mybir.AluOpType.add, op1=mybir.AluOpType.max)
                # residual + second elu
                r = epil.tile([128, D], F32, tag="r")
                nc.vector.tensor_add(r[:], el1[:], x03[:, ip * 8 + m, :])
                t2 = epil.tile([128, D], F32, tag="t2")
                nc.scalar.activation(t2[:], r[:],
                                     mybir.ActivationFunctionType.Relu,
                                     scale=-1.0)
                ex2 = epil.tile([128, D], F32, tag="ex2")
                nc.scalar.activation(ex2[:], t2[:],
                                     mybir.ActivationFunctionType.Exp,
                                     scale=-1.0)
                y = epil.tile([128, D], F32, tag="y")
                nc.vector.scalar_tensor_tensor(
                    out=y[:], in0=ex2[:], scalar=-1.0, in1=r[:],
                    op0=mybir.AluOpType.add, op1=mybir.AluOpType.max)
                row0 = (ip * 8 + m) * 128
                nc.sync.dma_start(out[row0:row0 + 128, :], y[:])

    nc.compile()
    return nc


def _get_nc():
    if "nc" not in _cache:
        _cache["nc"] = _build()
    return _cache["nc"]


def kernel(x0, adj0, W, a_src, a_dst):
    nc = _get_nc()
    in_maps = []
    for c in range(8):
        h, half = c // 2, c % 2
        i0 = half * NH
        a = adj0[h, i0:i0 + NH, :]
        if i0:
            a = np.concatenate([a[:, i0:], a[:, :i0]], axis=1)
            xr = np.concatenate([x0[i0:], x0[:i0]], axis=0)
        else:
            xr = x0
        in_maps.append(dict(
            adjT=np.ascontiguousarray(a.T).astype(ml_dtypes.bfloat16),
            x0r=np.ascontiguousarray(xr),
            w=np.ascontiguousarray(W[h]),
            asrc=np.ascontiguousarray(a_src[h][:, None]),
            adst=np.ascontiguousarray(a_dst[h][:, None]),
        ))
    res = run_bass_kernel_spmd(nc, in_maps, core_ids=list(range(8))).results
    x1 = np.empty((N, H * D), np.float32)
    for c in range(8):
        h, half = c // 2, c % 2
        i0 = half * NH
        x1[i0:i0 + NH, h * D:(h + 1) * D] = res[c]["out"]
    return x1
